# revision 1
# baseline (speedup 1.0000x reference)
"""SimCLR contrastive loss on 8 TRN2 NeuronCores.

Strategy (per spec sharding_hint): shard the N=8192 anchors row-wise across
8 cores; replicate the normalized pred/positive matrices. Normalization and
transposition are cheap O(N*D) host work; the O(N^2) similarity + exp +
row-reduction runs on device and never materializes the NxN matrices.

Host side (in kernel()):
  - L2-normalize rows of pred/positive (torch-style eps clamp).
  - s[i] = zp_i . zq_i  (positive-pair logit, exact diag of the pq matrix).
  - Build zpT/zqT = normalized matrices transposed to [D=128, N=8192], with
    columns rolled per core so each core's own 1024 anchor columns come
    first — the SPMD program is identical on all cores.

Device side (per core, identical program):
  - DMA zpT/zqT into SBUF as float32r (full-rate fp32 TensorEngine mode).
  - For each own 128-row chunk m: S-block = zpT[:, m-block].T @ Z*T against
    all 8192 columns (16 matmuls of [128,512] per matrix into PSUM).
  - ScalarE: exp(2*S) with accum_out => per-row partial sums, 2048 columns
    per ACTIVATE (4 PSUM banks). Only row sums leave the engine.
  - DMA out [128, 64] partial sums (8 m-chunks x 8 groups).

Host finish: neg_i = sum(partials_i) - e^2 (removes the pp diagonal,
exp(2*cos(x,x)) = e^2);  loss_i = log(neg_i) - 2*s_i;  mean over rows.
"""

import numpy as np

N = 8192
D = 128
P = 128
NCORES = 8
M_LOCAL = N // NCORES          # 1024 own rows per core
T_OWN = M_LOCAL // P           # 8 own row chunks
NB = 512                       # matmul moving free dim (one PSUM bank of f32)
GRP = 2048                     # columns per ACT exp instruction (4 banks)
N_GRP = N // GRP               # 4 groups per matrix per row-chunk
OUT_COLS = T_OWN * 2 * N_GRP   # 64 accum columns

EPS = 1e-8
TEMP = 0.5

_CACHE = {}


def _build_nc():
    import concourse.mybir as mybir
    from concourse import bacc
    from concourse.tile import TileContext
    from contextlib import ExitStack

    f32 = mybir.dt.float32
    f32r = mybir.dt.float32r
    AF = mybir.ActivationFunctionType

    nc = bacc.Bacc()
    zpt_d = nc.dram_tensor("zpt", [P, N], f32r, kind="ExternalInput")
    zqt_d = nc.dram_tensor("zqt", [P, N], f32r, kind="ExternalInput")
    out_d = nc.dram_tensor("out", [P, OUT_COLS], f32, kind="ExternalOutput")

    with TileContext(nc) as tc:
        with ExitStack() as ctx:
            sbuf = ctx.enter_context(tc.tile_pool(name="sbuf", bufs=1))
            zpT = sbuf.tile([P, N], f32r)
            zqT = sbuf.tile([P, N], f32r)
            outt = sbuf.tile([P, OUT_COLS], f32)

            # chunked loads so the first matmuls start after ~1 MiB lands
            for g in range(N_GRP):
                cs = slice(g * GRP, (g + 1) * GRP)
                nc.sync.dma_start(out=zpT[:, cs], in_=zpt_d[:, cs])
            for g in range(N_GRP):
                cs = slice(g * GRP, (g + 1) * GRP)
                nc.sync.dma_start(out=zqT[:, cs], in_=zqt_d[:, cs])

            ps_pool = ctx.enter_context(
                tc.tile_pool(name="ps_pool", bufs=2, space="PSUM"))
            scr_pool = ctx.enter_context(tc.tile_pool(name="scr_pool", bufs=2))
            # all pp row-chunks first (needs only zpt), then all pq — the
            # zqt DMA has the whole pp phase (~60us of ACT work) to land
            for mi, zT in enumerate((zpT, zqT)):
                for m in range(T_OWN):
                    lhsT = zpT[:, m * P:(m + 1) * P]
                    for g in range(N_GRP):
                        pt = ps_pool.tile([P, GRP], f32, tag="ps")
                        for s in range(GRP // NB):
                            col = g * GRP + s * NB
                            nc.tensor.matmul(
                                pt[:, s * NB:(s + 1) * NB],
                                lhsT=lhsT,
                                rhs=zT[:, col:col + NB],
                                start=True, stop=True,
                            )
                        scr = scr_pool.tile([P, GRP], f32, tag="scr")
                        acc_col = m * 8 + mi * N_GRP + g
                        nc.scalar.activation(
                            scr[:, :], pt[:, :], AF.Exp, scale=2.0,
                            accum_out=outt[:, acc_col:acc_col + 1],
                        )

            nc.sync.dma_start(out=out_d[:, :], in_=outt[:, :])

    nc.finalize()
    return nc


def _get_nc():
    if "nc" not in _CACHE:
        _CACHE["nc"] = _build_nc()
    return _CACHE["nc"]


def _host_prep(pred, positive):
    """Normalize rows, compute positive-pair logits, build transposed
    per-core (column-rolled) input matrices."""
    def nrm(x):
        n = np.sqrt(np.sum(x * x, axis=1, keepdims=True))
        return x / np.maximum(n, np.float32(EPS))

    zp = nrm(pred)
    zq = nrm(positive)
    s = np.sum(zp.astype(np.float64) * zq.astype(np.float64), axis=1)
    zpT = np.ascontiguousarray(zp.T)   # [D, N]
    zqT = np.ascontiguousarray(zq.T)
    return zpT, zqT, s


LAST_RESULTS = None


def kernel(pred: np.ndarray, positive: np.ndarray) -> np.ndarray:
    global LAST_RESULTS
    import sys
    if "/opt/trn_rl_repo" not in sys.path:
        sys.path.insert(0, "/opt/trn_rl_repo")
    from concourse.bass_utils import run_bass_kernel_spmd

    pred = np.ascontiguousarray(np.asarray(pred, dtype=np.float32))
    positive = np.ascontiguousarray(np.asarray(positive, dtype=np.float32))

    zpT, zqT, s = _host_prep(pred, positive)

    nc = _get_nc()
    in_maps = []
    for c in range(NCORES):
        k = c * M_LOCAL
        in_maps.append({
            "zpt": np.concatenate([zpT[:, k:], zpT[:, :k]], axis=1),
            "zqt": np.concatenate([zqT[:, k:], zqT[:, :k]], axis=1),
        })
    res = run_bass_kernel_spmd(nc, in_maps, core_ids=list(range(NCORES)))
    LAST_RESULTS = res

    # ---- unshard: combine per-core [128, 64] row-sum partials ----
    e2 = np.exp(np.float64(2.0))
    loss_sum = np.float64(0.0)
    for c in range(NCORES):
        o = np.asarray(res.results[c]["out"], dtype=np.float64)
        rowsum = o.reshape(P, T_OWN, 8).sum(axis=2)          # [p, m]
        neg = rowsum - e2
        # row (p, m) of core c is global row c*1024 + m*128 + p
        rows = (c * M_LOCAL
                + np.arange(T_OWN)[None, :] * P
                + np.arange(P)[:, None])
        loss_sum += np.sum(np.log(neg) - 2.0 * s[rows])
    return np.float32(loss_sum / N)

